# revision 1
# baseline (speedup 1.0000x reference)
"""MHA Bass kernel for TRN2, 8 NeuronCores (v2).

Sharding: data-parallel on batch (2) x tensor-parallel on heads (4 groups of 4
heads). Core c handles batch c//4 and head group c%4 (cols m0=256*(c%4)).

v2 layout: LayerNorm applied to x up front (stats via DVE/ACT accumulators,
xn = (x-mu)*r in one tensor_scalar), xn transposed once via DMA-xbar;
projections j-outer with PSUM tt-banks; attention restructured query-tile-outer
so the out-projection + f32 ReduceScatter for tile tt overlaps attention of
tile tt+1 (kills the serial collective tail); softmax denominators via
reciprocal_approx_fast; sigmoid via tanh (same ACT table set as exp).
"""
import numpy as np

B, LQ, D = 2, 2048, 1024
NHEAD, DHEAD = 16, 64
NC = 8
GPC = 4              # cores per batch group
MPC = 256            # output cols per core
N_DCH = D // 128     # 8 d-chunks
N_TCH = LQ // 128    # 16 token chunks
N_TT = LQ // 512     # 4 token tiles of 512

_NC_CACHE = [None]


def _build():
    import concourse.bacc as bacc
    import concourse.mybir as mybir
    from concourse import tile

    f32, bf16 = mybir.dt.float32, mybir.dt.bfloat16
    AF = mybir.ActivationFunctionType
    ALU = mybir.AluOpType

    nc = bacc.Bacc("TRN2", target_bir_lowering=False, debug=False, num_devices=NC)

    xq = nc.dram_tensor("xq", [LQ, D], f32, kind="ExternalInput").ap()
    xk = nc.dram_tensor("xk", [LQ, D], f32, kind="ExternalInput").ap()
    xv = nc.dram_tensor("xv", [LQ, D], f32, kind="ExternalInput").ap()
    wqT = nc.dram_tensor("wqT", [D, MPC], bf16, kind="ExternalInput").ap()
    wkT = nc.dram_tensor("wkT", [D, MPC], bf16, kind="ExternalInput").ap()
    wvT = nc.dram_tensor("wvT", [D, MPC], bf16, kind="ExternalInput").ap()
    wgT = nc.dram_tensor("wgT", [D, MPC], bf16, kind="ExternalInput").ap()
    woT = nc.dram_tensor("woT", [MPC, D], bf16, kind="ExternalInput").ap()
    bq_d = nc.dram_tensor("bq", [MPC], f32, kind="ExternalInput").ap()
    bk_d = nc.dram_tensor("bk", [MPC], f32, kind="ExternalInput").ap()
    bgh_d = nc.dram_tensor("bgh", [MPC], f32, kind="ExternalInput").ap()
    bvr_d = nc.dram_tensor("bvr", [1, MPC], bf16, kind="ExternalInput").ap()
    out_d = nc.dram_tensor("out", [N_TT, MPC, 512], f32, kind="ExternalOutput").ap()

    with tile.TileContext(nc) as tc:
        import contextlib
        es = contextlib.ExitStack()
        with es:
            const = es.enter_context(tc.tile_pool(name="const", bufs=1))
            persist = es.enter_context(tc.tile_pool(name="persist", bufs=1))

            ones = const.tile([128, 128], bf16)
            nc.gpsimd.memset(ones[:, :], 1.0)
            eps_t = const.tile([128, 1], f32)
            nc.gpsimd.memset(eps_t[:, :], 1e-5)

            wts = {}
            for nm, dr in (("q", wqT), ("k", wkT), ("v", wvT), ("g", wgT)):
                t = const.tile([128, N_DCH, MPC], bf16, tag=f"w{nm}")
                for j in range(N_DCH):
                    nc.sync.dma_start(out=t[:, j, :], in_=dr[128 * j:128 * (j + 1), :])
                wts[nm] = t
            wo_t = const.tile([128, 2, D], bf16)
            for mc in range(2):
                nc.sync.dma_start(out=wo_t[:, mc, :], in_=woT[128 * mc:128 * (mc + 1), :])
            biases = {}
            for nm, dr in (("q", bq_d), ("k", bk_d), ("g", bgh_d)):
                t = const.tile([128, 2], f32, tag=f"b{nm}")
                nc.sync.dma_start(out=t[:, :], in_=dr.rearrange("(c p) -> p c", p=128))
                biases[nm] = t
            bvr = const.tile([1, MPC], bf16)
            nc.sync.dma_start(out=bvr[:, :], in_=bvr_d[:, :])

            qhT = persist.tile([128, 2, LQ], bf16, tag="qhT")
            khT = persist.tile([128, 2, LQ], bf16, tag="khT")
            gT = persist.tile([128, 2, LQ], bf16, tag="gT")
            ygT = persist.tile([128, 2, LQ], bf16, tag="ygT")
            vaug = persist.tile([128, N_TCH, 4, 65], bf16, tag="vaug")
            nc.gpsimd.memset(vaug[:, :, :, :], 1.0)

            with tc.tile_pool(name="xrp", bufs=2) as xrp, \
                 tc.tile_pool(name="xtp", bufs=1) as xtp, \
                 tc.tile_pool(name="stp", bufs=2) as stp, \
                 tc.tile_pool(name="dmy", bufs=2) as dmy, \
                 tc.tile_pool(name="psA", bufs=2, space="PSUM") as psA, \
                 tc.tile_pool(name="psV", bufs=2, space="PSUM") as psV:

                def front(x_dram, projs, is_v):
                    xr = xrp.tile([128, N_TCH, D], bf16, tag="xr")
                    for qq in range(4):
                        nc.gpsimd.dma_start(
                            out=xr[:, 4 * qq:4 * (qq + 1), :],
                            in_=x_dram[512 * qq:512 * (qq + 1), :].rearrange(
                                "(i p) c -> p i c", p=128))
                    # stats: s1 = row sums (DVE), s2 = row sumsq (ACT)
                    s1 = stp.tile([128, N_TCH], f32, tag="s1")
                    s2 = stp.tile([128, N_TCH], f32, tag="s2")
                    for i in range(N_TCH):
                        d1 = dmy.tile([128, D], bf16, tag="d1")
                        nc.vector.tensor_scalar(
                            d1[:, :], xr[:, i, :], 1.0, 0.0,
                            op0=ALU.mult, op1=ALU.add, accum_out=s1[:, i:i + 1])
                        d2 = dmy.tile([128, D], bf16, tag="d2")
                        nc.scalar.activation(d2[:, :], xr[:, i, :], AF.Square,
                                             accum_out=s2[:, i:i + 1])
                    mu = stp.tile([128, N_TCH], f32, tag="mu")
                    nc.vector.tensor_scalar(mu[:, :], s1[:, :], 1.0 / D, None,
                                            op0=ALU.mult)
                    musq = stp.tile([128, N_TCH], f32, tag="musq")
                    nc.vector.tensor_tensor(musq[:, :], mu[:, :], mu[:, :],
                                            op=ALU.mult)
                    var = stp.tile([128, N_TCH], f32, tag="var")
                    nc.vector.scalar_tensor_tensor(
                        var[:, :], s2[:, :], 1.0 / D, musq[:, :],
                        op0=ALU.mult, op1=ALU.subtract)
                    sd = stp.tile([128, N_TCH], f32, tag="sd")
                    nc.scalar.activation(sd[:, :], var[:, :], AF.Sqrt,
                                         bias=eps_t[:, 0:1])
                    r_t = stp.tile([128, N_TCH], f32, tag="r")
                    nc.vector.reciprocal_approx_fast(r_t[:, :], sd[:, :])
                    # normalize in place, then transpose
                    xnT = xtp.tile([128, N_TCH, N_DCH, 128], bf16, tag="xnT")
                    for i in range(N_TCH):
                        nc.vector.tensor_scalar(
                            xr[:, i, :], xr[:, i, :], mu[:, i:i + 1], r_t[:, i:i + 1],
                            op0=ALU.subtract, op1=ALU.mult)
                        nc.sync.dma_start(out=xnT[:, i, :, :], in_=xr[:, i, :],
                                          transpose=True)

                    if is_v:
                        # token-major v: lhsT = xnT block, rhs = wvT -> [tok, m]
                        w = wts["v"]
                        for s in range(N_TCH):
                            pv = psV.tile([128, MPC], f32, tag="pV")
                            for j in range(N_DCH):
                                nc.tensor.matmul(pv[:, :], xnT[:, s, j, :],
                                                 w[:, j, :], start=(j == 0),
                                                 stop=False)
                            nc.tensor.matmul(pv[:, :], ones[0:1, :], bvr[:, :],
                                             start=False, stop=True)
                            nc.vector.tensor_copy(
                                vaug[:, s, :, 0:64],
                                pv[:, :].rearrange("p (h c) -> p h c", h=4))
                        return

                    for nm, out_t in projs:
                        w = wts[nm]
                        sigm = nm == "g"
                        bias = biases[nm]
                        for mc in range(2):
                            for tth in range(2):
                                pp = psA.tile([128, 2, 512], f32, tag="pA")
                                for j in range(N_DCH):
                                    for t2 in range(2):
                                        tt = 2 * tth + t2
                                        nc.tensor.matmul(
                                            pp[:, t2, :],
                                            w[:, j, 128 * mc:128 * (mc + 1)],
                                            xnT[:, 4 * tt:4 * (tt + 1), j, :],
                                            start=(j == 0), stop=(j == N_DCH - 1))
                                for t2 in range(2):
                                    sl = slice(512 * (2 * tth + t2),
                                               512 * (2 * tth + t2 + 1))
                                    if sigm:
                                        gp = dmy.tile([128, 512], bf16, tag="gp")
                                        nc.scalar.activation(
                                            gp[:, :], pp[:, t2, :], AF.Tanh,
                                            bias=bias[:, mc:mc + 1], scale=0.5)
                                        nc.vector.tensor_scalar(
                                            out_t[:, mc, sl], gp[:, :], 0.5, 0.5,
                                            op0=ALU.mult, op1=ALU.add)
                                    else:
                                        nc.vector.tensor_scalar(
                                            out_t[:, mc, sl], pp[:, t2, :],
                                            bias[:, mc:mc + 1], None, op0=ALU.add)

                front(xq, [("q", qhT), ("g", gT)], False)
                front(xk, [("k", khT)], False)
                front(xv, None, True)

            # ---- attention, query-tile outer; out-proj + RS per tile ----
            with tc.tile_pool(name="att", bufs=2) as att, \
                 tc.tile_pool(name="od", bufs=4) as od, \
                 tc.tile_pool(name="ps_st", bufs=2, space="PSUM") as ps_st, \
                 tc.tile_pool(name="ps_o", bufs=2, space="PSUM") as ps_o, \
                 tc.tile_pool(name="ps_po", bufs=2, space="PSUM") as ps_po, \
                 tc.tile_pool(name="dram", bufs=4, space="DRAM") as dram_p:
                for tt in range(N_TT):
                    sl = slice(512 * tt, 512 * (tt + 1))
                    for hp in range(2):
                        o_ps = [ps_o.tile([65, 512], f32, name=f"o{hb}", tag="o")
                                for hb in range(2)]
                        for s in range(N_TCH):
                            st = ps_st.tile([128, 1024], f32, tag="st")
                            for hb in range(2):
                                r0 = 64 * hb
                                nc.tensor.matmul(
                                    st[:, 512 * hb:512 * (hb + 1)],
                                    khT[r0:r0 + 64, hp, 128 * s:128 * (s + 1)],
                                    qhT[r0:r0 + 64, hp, sl],
                                    start=True, stop=True)
                            pt = att.tile([128, 1024], bf16, tag="pt")
                            nc.scalar.activation(pt[:, :], st[:, :], AF.Exp,
                                                 scale=0.125)
                            for hb in range(2):
                                nc.tensor.matmul(
                                    o_ps[hb][:, :], vaug[:, s, 2 * hp + hb, :],
                                    pt[:, 512 * hb:512 * (hb + 1)],
                                    start=(s == 0), stop=(s == N_TCH - 1))
                        for hb in range(2):
                            o_p = o_ps[hb]
                            # broadcast denominators l across 64 partitions,
                            # then reciprocal on the full [64, 512] tile
                            lib = att.tile([65, 512], bf16, tag="lib")
                            nc.vector.tensor_copy(lib[64:65, :], o_p[64:65, :])
                            bc = ps_po.tile([128, 512], f32, tag="po")
                            nc.tensor.matmul(bc[0:64, :], ones[64:65, 0:64],
                                             lib[64:65, :], start=True, stop=True)
                            bcl = att.tile([64, 512], f32, tag="bcl")
                            nc.vector.tensor_copy(bcl[:, :], bc[0:64, :])
                            bcs = att.tile([64, 512], f32, tag="bcs")
                            nc.vector.reciprocal_approx_fast(bcs[:, :], bcl[:, :])
                            yt = att.tile([128, 512], f32, tag="yt")
                            nc.vector.tensor_mul(yt[0:64, :], o_p[0:64, :],
                                                 bcs[:, :])
                            r0 = 64 * hb
                            if hb == 0:
                                nc.vector.tensor_mul(ygT[0:64, hp, sl],
                                                     yt[0:64, :], gT[0:64, hp, sl])
                            else:
                                nc.gpsimd.dma_start(out=yt[64:128, :],
                                                    in_=yt[0:64, :])
                                nc.vector.tensor_mul(ygT[64:128, hp, sl],
                                                     yt[64:128, :],
                                                     gT[64:128, hp, sl])
                    # out-projection for this token tile + overlapped RS
                    outb = dram_p.tile([D, 512], f32, tag="outb")
                    for nk in range(N_DCH):
                        po = ps_po.tile([128, 512], f32, tag="po")
                        for mc in range(2):
                            nc.tensor.matmul(po[:, :],
                                             wo_t[:, mc, 128 * nk:128 * (nk + 1)],
                                             ygT[:, mc, sl],
                                             start=(mc == 0), stop=(mc == 1))
                        ot = od.tile([128, 512], f32, tag="ot")
                        nc.vector.tensor_copy(ot[:, :], po[:, :])
                        nc.sync.dma_start(out=outb[128 * nk:128 * (nk + 1), :],
                                          in_=ot[:, :])
                    outrs = dram_p.tile([MPC, 512], f32, tag="outrs")
                    nc.gpsimd.collective_compute(
                        "ReduceScatter", ALU.add,
                        replica_groups=[[0, 1, 2, 3], [4, 5, 6, 7]],
                        ins=[outb[:, :].opt()],
                        outs=[outrs[:, :].opt()])
                    nc.sync.dma_start(out=out_d[tt, :, :], in_=outrs[:, :])

    nc.compile()
    return nc


def kernel(q, k, v, qln_g, qln_b, kvln_g, kvln_b, Wq, Wk, Wv, Wg, bg, Wo):
    import concourse.mybir as mybir
    from concourse import bass_utils

    bf16 = mybir.dt.np(mybir.dt.bfloat16)
    q = np.asarray(q, np.float32)
    k = np.asarray(k, np.float32)
    v = np.asarray(v, np.float32)
    qln_g = np.asarray(qln_g, np.float32)
    qln_b = np.asarray(qln_b, np.float32)
    kvln_g = np.asarray(kvln_g, np.float32)
    kvln_b = np.asarray(kvln_b, np.float32)
    Wq, Wk, Wv = np.asarray(Wq, np.float32), np.asarray(Wk, np.float32), np.asarray(Wv, np.float32)
    Wg, Wo = np.asarray(Wg, np.float32), np.asarray(Wo, np.float32)
    bg = np.asarray(bg, np.float32)

    # fold LN gamma into weights; beta into bias vectors
    Wqp, Wgp = Wq * qln_g[None, :], Wg * qln_g[None, :]
    Wkp, Wvp = Wk * kvln_g[None, :], Wv * kvln_g[None, :]
    bq_f, bk_f, bv_f = Wq @ qln_b, Wk @ kvln_b, Wv @ kvln_b
    bg_f = Wg @ qln_b + bg

    if _NC_CACHE[0] is None:
        _NC_CACHE[0] = _build()
    nc = _NC_CACHE[0]

    in_maps = []
    for c in range(NC):
        beta, g = c // GPC, c % GPC
        m0 = MPC * g
        sl = slice(m0, m0 + MPC)
        in_maps.append({
            "xq": q[beta], "xk": k[beta], "xv": v[beta],
            "wqT": Wqp[sl, :].T.astype(bf16), "wkT": Wkp[sl, :].T.astype(bf16),
            "wvT": Wvp[sl, :].T.astype(bf16), "wgT": Wgp[sl, :].T.astype(bf16),
            "woT": Wo[:, sl].T.astype(bf16),
            "bq": bq_f[sl], "bk": bk_f[sl], "bgh": 0.5 * bg_f[sl],
            "bvr": bv_f[sl][None, :].astype(bf16),
        })
    global _last_in_maps
    _last_in_maps = in_maps
    res = bass_utils.run_bass_kernel_spmd(nc, in_maps, core_ids=list(range(NC)))
    out = np.empty((B, LQ, D), np.float32)
    for beta in range(B):
        for g in range(GPC):
            r = res.results[GPC * beta + g]["out"]  # [4, 256, 512]
            for tt in range(N_TT):
                out[beta, 512 * tt:512 * (tt + 1), MPC * g:MPC * (g + 1)] = r[tt].T
    return out

